# revision 9
# baseline (speedup 1.0000x reference)
"""Self-contained TRN2 Bass kernel for nn_CAM_Module (channel attention).

kernel(x, gamma): x [16,512,64,64] f32, gamma [1] f32 -> [16,512,64,64] f32.
Data-parallel over batch: 2 samples per NeuronCore across 8 cores.

Math: q = x.reshape(B,C,HW); E = q@q.T; softmax(rowmax(E)-E) == softmax(-E)
(shift invariance), computed as exp(rowmin(E)-E)/rowsum; out = gamma*(A@q)+x.

Per-core pipeline (2 samples):
  - fp32 pieces DMA'd in (sized for 2-8KB DMA packets), cast to fp16 on
    DVE/ACT; q^T built 2-chunk groups via PE transposes (fp16 PSUM
    bounce -> one fp16 SBUF copy).
  - Gram accumulated in fp32 PSUM, upper-triangle blocks at natural
    offsets; lower blocks mirrored by fp32 PE transposes directly into
    the PSUM holes; rowmin/exp read PSUM in place.
  - softmax folded into the attention operand: lhsT blocks are built as
    diag(gamma/Z) @ exp tiles via matmuls with a diagonal rhs, plus an
    identity accumulate on diagonal blocks, so the A-matmul emits the
    final  gamma*softmax@q + x  directly; epilogue is a single
    f32->f16 copy and the output tensor is stored f16 (halves output
    HBM traffic; host converts back to f32).
  - schedule interleaves sample-0's A-matmul with sample-1's Gram so the
    PE stays fed during sample-1's load; softmax latencies are covered
    by reserved A-chunks.
"""
import sys
if '/opt/trn_rl_repo' not in sys.path:
    sys.path.insert(0, '/opt/trn_rl_repo')
import numpy as np
import concourse.bass as bass
import concourse.tile as tile
import concourse.mybir as mybir
from concourse.masks import make_identity

F32 = mybir.dt.float32
F16 = mybir.dt.float16

C = 512          # channels
N = 4096         # spatial (64*64)
CB = C // 128    # 4 c-blocks
NK = N // 128    # 32 transpose chunks
NG = NK // 2     # 16 transpose groups (2 chunks per PSUM bounce bank)
S = 2            # samples per core
# input pieces per (sample, c-block): (col offset, width). Widths chosen
# so later DMAs move 4-8KB per-row packets while the first pieces land
# fast enough to start compute early.
PIECES = [(0, 512), (512, 512), (1024, 1024), (2048, 1024), (3072, 1024)]
# upper-triangle mirror slots: (row-block j, col-block i), j < i
UPPER = [(0, 1), (0, 2), (0, 3), (1, 2), (1, 3), (2, 3)]


def build(nc: bass.Bass):
    x_ext = nc.declare_dram_parameter("x", [S * C, N], F32, isOutput=False)
    g_ext = nc.declare_dram_parameter("gamma", [1, 1], F32, isOutput=False)
    out_ext = nc.declare_dram_parameter("out", [S * C, N], F16, isOutput=True)
    x_ap = x_ext.ap()
    out_ap = out_ext.ap()

    with tile.TileContext(nc) as tc:
        with (
            tc.tile_pool(name="const", bufs=1) as const,
            tc.tile_pool(name="x32", bufs=1) as xpool,
            tc.tile_pool(name="q16", bufs=8) as q16p,
            tc.tile_pool(name="qt", bufs=8) as qtp,
            tc.tile_pool(name="esb", bufs=2) as esbp,
            tc.tile_pool(name="expn", bufs=2) as expnp,
            tc.tile_pool(name="expt", bufs=8) as exptp,
            tc.tile_pool(name="vecs", bufs=8) as vecs,
            tc.tile_pool(name="outs", bufs=4) as outsp,
            tc.tile_pool(name="ps_t", bufs=2, space="PSUM") as ps_t,
            tc.tile_pool(name="ps_e", bufs=1, space="PSUM") as ps_e,
            tc.tile_pool(name="ps_o", bufs=2, space="PSUM") as ps_o,
        ):
            ident16 = const.tile([128, 128], F16)
            make_identity(nc, ident16)
            ident32 = const.tile([128, 128], F32)
            make_identity(nc, ident32)
            gbc = const.tile([128, 1], F32)
            nc.gpsimd.dma_start(out=gbc, in_=g_ext.ap().to_broadcast((128, 1)))

            st = [dict(q16=[None] * CB, x32={}, cast_done=set(),
                       qtc={}, expT=[None] * CB, diag=[None] * CB)
                  for _ in range(S)]

            cast_cyc = [nc.vector, nc.scalar, nc.gpsimd]
            cast_i = [0]
            evac_cyc = [nc.vector, nc.scalar]
            evac_i = [0]
            acpy_cyc = [nc.vector, nc.scalar, nc.vector]

            def cp(eng, out, in_):
                if eng is nc.scalar:
                    nc.scalar.copy(out, in_)
                else:
                    eng.tensor_copy(out, in_)

            def submit_loads(s):
                for cb in range(CB):
                    q = q16p.tile([128, N], F16, tag="q16",
                                  name=f"q16_{s}_{cb}")
                    st[s]["q16"][cb] = q
                for pi, (off, wdt) in enumerate(PIECES):
                    for cb in range(CB):
                        xt = xpool.tile([128, wdt], F32, tag=f"xt{pi}",
                                        bufs=4,
                                        name=f"xt_{s}_{cb}_{pi}")
                        nc.sync.dma_start(
                            out=xt,
                            in_=x_ap[
                                s * C + cb * 128 : s * C + (cb + 1) * 128,
                                off : off + wdt,
                            ],
                        )
                        st[s]["x32"][(cb, pi)] = (xt, off, wdt)

            def ensure_cast(s, w):
                # emit fp32->fp16 casts for all 512-col windows up to w
                for wi in range(w + 1):
                    if wi in st[s]["cast_done"]:
                        continue
                    st[s]["cast_done"].add(wi)
                    base = wi * 512
                    pi = next(i for i, (o, wd) in enumerate(PIECES)
                              if o <= base < o + wd)
                    for cb in range(CB):
                        xt, off, _ = st[s]["x32"][(cb, pi)]
                        eng = cast_cyc[cast_i[0] % len(cast_cyc)]
                        cast_i[0] += 1
                        cp(eng,
                           st[s]["q16"][cb][:, base : base + 512],
                           xt[:, base - off : base - off + 512])

            def t_ops(s, g):
                # generator: 8 transposes of group g into a PSUM bounce
                bounce = ps_t.tile([128, 2, CB, 128], F16, tag="bounce",
                                   name=f"bounce_{s}_{g}")
                for cb in range(CB):
                    for h in range(2):
                        k = 2 * g + h
                        yield lambda cb=cb, h=h, k=k, b=bounce: (
                            nc.tensor.transpose(
                                b[:, h, cb, :],
                                st[s]["q16"][cb][:, k * 128 : (k + 1) * 128],
                                ident16,
                            ))
                st[s]["bounce_" + str(g)] = bounce

            def t_evac(s, g):
                bounce = st[s].pop("bounce_" + str(g))
                qtc = qtp.tile([128, 2, CB, 128], F16, tag="qtc",
                               name=f"qtc_{s}_{g}")
                eng = evac_cyc[evac_i[0] % 2]
                evac_i[0] += 1
                cp(eng, qtc[:], bounce[:])
                st[s]["qtc"][g] = qtc

            def e_ops(s, g):
                # generator: 8 Gram matmuls of group g (upper-tri blocks)
                if "E" not in st[s]:
                    st[s]["E"] = ps_e.tile([128, CB, 512], F32, tag="E",
                                           name=f"E_{s}")
                E = st[s]["E"]
                qtc = st[s]["qtc"].pop(g)
                for h in range(2):
                    k = 2 * g + h
                    for m in range(CB):
                        yield lambda h=h, k=k, m=m, q=qtc: (
                            nc.tensor.matmul(
                                E[:, m, m * 128 : 512],
                                lhsT=q[:, h, m, :],
                                rhs=q[:, h, m:CB, :],
                                start=(k == 0),
                                stop=(k == NK - 1),
                            ))

            def tgroup(s, g):
                for op in t_ops(s, g):
                    op()
                t_evac(s, g)

            def emm(s, g):
                for op in e_ops(s, g):
                    op()

            def weave(s, g_t, s_e, g_e):
                # alternate Gram matmuls of group g_e with transposes of
                # group g_t so each transpose's stationary load hides
                # under the preceding matmul's stream
                ts = list(t_ops(s, g_t))
                ms = list(e_ops(s_e, g_e))
                for m_op, t_op in zip(ms, ts):
                    m_op()
                    t_op()
                t_evac(s, g_t)

            def softmax_head(s):
                # mirror lower triangle into E's PSUM holes, then
                # rowmin + exp (with fused rowsum) reading PSUM in place
                E = st[s]["E"]
                esb = esbp.tile([128, 6, 128], F32, tag="esb",
                                name=f"esb_{s}")
                cp(nc.vector, esb[:, 0:3, :], E[:, 0, 128:512])
                cp(nc.scalar, esb[:, 3:5, :], E[:, 1, 256:512])
                cp(nc.vector, esb[:, 5:6, :], E[:, 2, 384:512])
                for slot, (j, i) in enumerate(UPPER):
                    nc.tensor.transpose(
                        E[:, i, j * 128 : (j + 1) * 128],
                        esb[:, slot, :],
                        ident32,
                    )
                expn = expnp.tile([128, CB, 512], F16, tag="expn",
                                  name=f"expn_{s}")
                Zs = []
                for m in range(CB):
                    mv = vecs.tile([128, 1], F32, tag="mv", name=f"mv_{s}_{m}")
                    nc.vector.tensor_reduce(
                        mv, E[:, m, :], axis=mybir.AxisListType.X,
                        op=mybir.AluOpType.min,
                    )
                    Z = vecs.tile([128, 1], F32, tag="Z", name=f"Z_{s}_{m}")
                    nc.scalar.activation(
                        expn[:, m, :],
                        E[:, m, :],
                        mybir.ActivationFunctionType.Exp,
                        bias=mv,
                        scale=-1.0,
                        accum_out=Z,
                    )
                    Zs.append(Z)
                st[s]["expn"] = expn
                st[s]["Zs"] = Zs

            def softmax_tail(s):
                # diag(gamma/Z) tiles: fold softmax normalization and
                # gamma into the A-matmul's lhsT
                for m in range(CB):
                    rz = vecs.tile([128, 1], F32, tag="rz", name=f"rz_{s}_{m}")
                    nc.vector.reciprocal(rz, st[s]["Zs"][m])
                    sc = vecs.tile([128, 1], F32, tag="sc", name=f"sc_{s}_{m}")
                    nc.vector.tensor_mul(sc, rz, gbc)  # gamma / Z
                    dg = vecs.tile([128, 128], F16, tag="diag",
                                   name=f"diag_{s}_{m}")
                    nc.vector.tensor_scalar_mul(dg, ident16, sc)
                    st[s]["diag"][m] = dg

            def expT(s):
                # lhsT blocks for the A-matmul: exp^T scaled by
                # diag(gamma/Z), plus identity on diagonal blocks so the
                # matmul emits gamma*A@q + q directly
                expn = st[s]["expn"]
                for j in range(CB):
                    eb = ps_t.tile([128, CB, 128], F32, tag="bounce",
                                   name=f"ebounce_{s}_{j}")
                    for cb in range(CB):
                        nc.tensor.matmul(
                            eb[:, cb, :],
                            lhsT=expn[:, cb, j * 128 : (j + 1) * 128],
                            rhs=st[s]["diag"][cb],
                            start=True,
                            stop=(cb != j),
                        )
                        if cb == j:
                            nc.tensor.matmul(
                                eb[:, cb, :],
                                lhsT=ident16,
                                rhs=ident16,
                                start=False,
                                stop=True,
                            )
                    et = exptp.tile([128, CB, 128], F16, tag="expT",
                                    name=f"expT_{s}_{j}")
                    eng = evac_cyc[evac_i[0] % 2]
                    evac_i[0] += 1
                    cp(eng, et[:], eb[:])
                    st[s]["expT"][j] = et

            def achunk(s, cb, no, copy_engs, rotate=False):
                # final out chunk: gamma*(A@q)+x lands in PSUM directly;
                # epilogue is one f32->f16 copy into the staging tile.
                # rotate=True also cycles through the (idle) bounce pool
                # for a 4-deep acc pipeline during the pure-A phase.
                acopy_i[0] += 1
                if rotate and acopy_i[0] % 2 == 0:
                    acc = ps_t.tile([128, 512], F32, tag="bounce",
                                    name=f"acc_{s}_{cb}_{no}")
                else:
                    acc = ps_o.tile([128, 512], F32, tag="acc",
                                    name=f"acc_{s}_{cb}_{no}")
                for j in range(CB):
                    nc.tensor.matmul(
                        acc[:],
                        lhsT=st[s]["expT"][j][:, cb, :],
                        rhs=st[s]["q16"][j][:, no * 512 : (no + 1) * 512],
                        start=(j == 0),
                        stop=(j == CB - 1),
                    )
                key = (s, cb, no // 4)
                if key not in ostage:
                    ostage[key] = outsp.tile([128, 2048], F16, tag="ot",
                                             name=f"ot_{s}_{cb}_{no // 4}")
                ot = ostage[key]
                eng = copy_engs[acopy_i[0] % len(copy_engs)]
                cp(eng, ot[:, (no % 4) * 512 : (no % 4) * 512 + 512], acc[:])
                last = (s == 1 and cb == CB - 1 and no >= 4)
                if last and no % 2 == 1:
                    h = (no % 4) // 2
                    nc.sync.dma_start(
                        out=out_ap[
                            s * C + cb * 128 : s * C + (cb + 1) * 128,
                            (no // 4) * 2048 + h * 1024 :
                            (no // 4) * 2048 + (h + 1) * 1024,
                        ],
                        in_=ot[:, h * 1024 : (h + 1) * 1024],
                    )
                elif not last and no % 4 == 3:
                    nc.sync.dma_start(
                        out=out_ap[
                            s * C + cb * 128 : s * C + (cb + 1) * 128,
                            (no // 4) * 2048 : (no // 4 + 1) * 2048,
                        ],
                        in_=ot[:],
                    )

            ostage = {}
            acopy_i = [0]

            # ---- emission schedule -----------------------------------
            submit_loads(0)
            submit_loads(1)

            # sample-0 Gram, DMA-paced; transposes of group g woven
            # between group g-2's Gram matmuls so each group's bounce
            # evacuation overlaps a full woven group (no PE stall at the
            # group boundary)
            ensure_cast(0, 0)
            tgroup(0, 0)
            tgroup(0, 1)
            for g in range(2, NG):
                ensure_cast(0, g // 2)
                weave(0, g, 0, g - 2)
            emm(0, NG - 2)
            emm(0, NG - 1)

            # softmax0; its exp/rowmin latency is covered by sample-1's
            # first transpose groups (no emm yet: E bank busy). The
            # diag-build ops are emitted first so they don't queue
            # behind the cover evacs on DVE.
            softmax_head(0)
            softmax_tail(0)
            for g in range(8):
                ensure_cast(1, g // 2)
                tgroup(1, g)
            expT(0)

            # main interleave: sample-1 Gram paced by its load, sample-0
            # A-chunks fill the PE slack; outputs start draining early
            aq = [(cb, no) for cb in range(CB) for no in range(8)]
            ai = 0
            for g in range(8, NG):
                ensure_cast(1, g // 2)
                weave(1, g, 1, g - 8)
                achunk(0, *aq[ai], acpy_cyc)
                ai += 1
                achunk(0, *aq[ai], acpy_cyc)
                ai += 1
            for g in range(8, NG):
                emm(1, g)
                achunk(0, *aq[ai], acpy_cyc)
                ai += 1

            # softmax1; exp latency covered by remaining a0 chunks
            softmax_head(1)
            softmax_tail(1)
            for _ in range(8):
                achunk(0, *aq[ai], [nc.vector])
                ai += 1
            expT(1)

            for cb in range(CB):
                for no in range(8):
                    achunk(1, cb, no, acpy_cyc, rotate=True)
    return nc


def _split_excess_waits(nc, max_waits=1):
    """This container's walrus rejects >1 sync-wait on one instruction
    ("Too many sync wait commands"); hoist extras onto standalone
    InstEventSemaphore preludes on the same engine."""
    n = 0
    for fn in nc.m.functions:
        for bb in fn.blocks:
            out = []
            for inst in bb.instructions:
                si = inst.sync_info
                if si is not None and si.on_wait and len(si.on_wait) > max_waits:
                    waits = list(si.on_wait)
                    head, keep = waits[:-max_waits], waits[-max_waits:]
                    for i, w in enumerate(head):
                        ev = mybir.InstEventSemaphore(
                            name=f"{inst.name}-wsplit{i}", ins=[], outs=[])
                        ev.engine = inst.engine
                        ev.sync_info = mybir.SyncInfo(on_wait=[w], on_update=[])
                        out.append(ev)
                        n += 1
                    inst.sync_info = mybir.SyncInfo(
                        on_wait=keep, on_update=list(si.on_update))
                out.append(inst)
            bb.instructions[:] = out
    return n


_cache = {}


def _get_nc():
    if 'nc' not in _cache:
        nc = bass.Bass()
        build(nc)
        _split_excess_waits(nc)
        _cache['nc'] = nc
    return _cache['nc']


def kernel(x: np.ndarray, gamma: np.ndarray) -> np.ndarray:
    from concourse.bass_utils import run_bass_kernel_spmd

    B, CH, H, W = x.shape          # (16, 512, 64, 64)
    NSP = H * W
    M = 8                          # cores
    SS = B // M                    # samples per core
    nc = _get_nc()
    g = np.ascontiguousarray(gamma, dtype=np.float32).reshape(1, 1)
    in_maps = [
        {
            "x": np.ascontiguousarray(
                x[i * SS : (i + 1) * SS].reshape(SS * CH, NSP), dtype=np.float32
            ),
            "gamma": g,
        }
        for i in range(M)
    ]
    res = run_bass_kernel_spmd(nc, in_maps, core_ids=list(range(M)))
    out = np.concatenate(
        [res.results[i]["out"].astype(np.float32).reshape(SS, CH, H, W)
         for i in range(M)],
        axis=0,
    )
    return np.ascontiguousarray(out, dtype=np.float32)
